# revision 1
# baseline (speedup 1.0000x reference)
"""Tensor-parallel causal GQA self-attention (B=1, S=2048, D=4096, 32 q heads /
8 kv heads, HD=128, interleaved RoPE) on 8 trn2 NeuronCores.

Sharding: core c owns kv head c and q heads 4c..4c+3 (column-parallel
Wq/Wk/Wv, row-parallel Wo).  Each core computes a full [S, D] partial of the
output projection; the host sums the 8 partials (the "all-reduce").

On-device layout strategy (all matmuls fp32r = fp22 multiply at full PE rate):
  QT/KT  [HD, S]   "transposed" per-head layouts straight out of the
                   projection matmuls (lhsT = W d-tile, rhs = xT d-tile)
  RoPE   applied in [HD, S] layout: rope(q) = q*C2 + perm(q)*S2, where
                   perm is a 128x128 pair-swap matmul and C2/S2 host tables
  scores ST[j, i] = KT_tile.T @ QT_chunk  (single K=128 pass, N=512)
  P      exp(ST + causal_mask) via ScalarE, masks host-precomputed
  AV     OT[h, i] += V_tile.T(natural) @ PT_tile  (fp32r, N=512)
  denom  DVE-accumulated sum of PT tiles, partition-reduced by a ones-column
         matmul, reciprocal, broadcast back to 128 partitions by a K=1 matmul
  o_proj out[s, dout] += OTn_tile.T @ Wo_tile  (fp32r, N=512)
"""

import sys

if "/opt/trn_rl_repo" not in sys.path:
    sys.path.insert(0, "/opt/trn_rl_repo")

import numpy as np

import concourse.bass as bass
import concourse.tile as tile
from concourse import bacc, mybir
from concourse.bass_utils import run_bass_kernel_spmd

S, D, NH, NKV, HD = 2048, 4096, 32, 8, 128
NCORES = 8
QH = NH // NCORES  # 4 q heads per core
ROPE_BASE = 500000.0
NEG = -1e30

F32 = mybir.dt.float32
F32R = mybir.dt.float32r
AF = mybir.ActivationFunctionType

SC = S // 512  # 4 s-chunks of 512
DT = D // 128  # 32 d-tiles of 128
JT = S // 128  # 16 j-tiles of 128

_CACHE = {}

# set by test harness to collect an exec-time profile
TRACE = False
LAST_EXEC_NS = None


def _build_nc():
    nc = bacc.Bacc("TRN2", target_bir_lowering=False, debug=False,
                   num_devices=NCORES)

    xT_d = nc.declare_dram_parameter("xT", [D, S], F32R, isOutput=False)
    wq_d = nc.declare_dram_parameter("wq", [D, QH * HD], F32R, isOutput=False)
    wkv_d = nc.declare_dram_parameter("wkv", [D, 2 * HD], F32R, isOutput=False)
    wo_d = nc.declare_dram_parameter("wo", [QH * HD, D], F32R, isOutput=False)
    cos_d = nc.declare_dram_parameter("cos2", [HD, S], F32, isOutput=False)
    sin_d = nc.declare_dram_parameter("sin2", [HD, S], F32, isOutput=False)
    perm_d = nc.declare_dram_parameter("perm", [HD, HD], F32R, isOutput=False)
    masks_d = nc.declare_dram_parameter("masks", [4, 128, 512], F32,
                                        isOutput=False)
    onr_d = nc.declare_dram_parameter("ones_red", [128, 1], F32R,
                                      isOutput=False)
    onb_d = nc.declare_dram_parameter("ones_bc", [1, 128], F32R,
                                      isOutput=False)
    ident_d = nc.declare_dram_parameter("ident", [HD, HD], F32R,
                                        isOutput=False)
    out_d = nc.declare_dram_parameter("out", [S, D], F32, isOutput=True)

    with tile.TileContext(nc) as tc:
        from contextlib import ExitStack
        ctx = ExitStack()
        with ctx:
            wpool = ctx.enter_context(tc.tile_pool(name="wpool", bufs=32))
            wkvp = ctx.enter_context(tc.tile_pool(name="wkvp", bufs=4))
            xpool = ctx.enter_context(tc.tile_pool(name="xpool", bufs=4))
            headp = ctx.enter_context(tc.tile_pool(name="headp", bufs=5))
            tabp = ctx.enter_context(tc.tile_pool(name="tabp", bufs=2))
            ktp = ctx.enter_context(tc.tile_pool(name="ktp", bufs=1))
            vnp = ctx.enter_context(tc.tile_pool(name="vnp", bufs=1))
            stg = ctx.enter_context(tc.tile_pool(name="stg", bufs=5))
            ptp = ctx.enter_context(tc.tile_pool(name="ptp", bufs=3))
            dap = ctx.enter_context(tc.tile_pool(name="dap", bufs=2))
            mkp = ctx.enter_context(tc.tile_pool(name="mkp", bufs=4))
            cst = ctx.enter_context(tc.tile_pool(name="cst", bufs=1))
            rcp = ctx.enter_context(tc.tile_pool(name="rcp", bufs=2))
            psA = ctx.enter_context(
                tc.tile_pool(name="psA", bufs=6, space=bass.MemorySpace.PSUM))
            psB = ctx.enter_context(
                tc.tile_pool(name="psB", bufs=2, space=bass.MemorySpace.PSUM))

            # ---- constants / tables ----
            perm_t = cst.tile([HD, HD], F32R, name="perm_t")
            nc.sync.dma_start(perm_t[:], perm_d[:])
            ident_t = cst.tile([HD, HD], F32R, name="ident_t")
            nc.sync.dma_start(ident_t[:], ident_d[:])
            onr_t = cst.tile([128, 1], F32R, name="onr_t")
            nc.sync.dma_start(onr_t[:], onr_d[:])
            onb_t = cst.tile([1, 128], F32R, name="onb_t")
            nc.sync.dma_start(onb_t[:], onb_d[:])
            cos_t = tabp.tile([HD, S], F32, tag="tab", name="cos_t")
            nc.sync.dma_start(cos_t[:], cos_d[:])
            sin_t = tabp.tile([HD, S], F32, tag="tab", name="sin_t")
            nc.sync.dma_start(sin_t[:], sin_d[:])
            mask_t = []
            for m in range(4):
                mt = mkp.tile([128, 512], F32, tag="mask", name=f"mask{m}")
                nc.sync.dma_start(mt[:], masks_d[m])
                mask_t.append(mt)

            # Wq resident: 32 d-tiles of [128, 512]
            wq_tiles = []
            for d_i in range(DT):
                wt = wpool.tile([128, QH * HD], F32R, tag="w",
                                name=f"wq{d_i}")
                nc.sync.dma_start(wt[:], wq_d[128 * d_i:128 * (d_i + 1), :])
                wq_tiles.append(wt)

            # persistent activations
            QT = [headp.tile([HD, S], F32R, tag="hb", name=f"qt{h}")
                  for h in range(QH)]
            KT = ktp.tile([HD, S], F32R, name="kt")
            Vn = vnp.tile([128, S], F32R, name="vn")

            # ---- phase 1: QKV projections + RoPE + V transpose ----
            def rope(acc_ps, dest, sc):
                raw = stg.tile([128, 512], F32R, tag="stg", name="rope_raw")
                nc.any.tensor_copy(raw[:], acc_ps[:])
                rot = psB.tile([128, 512], F32, tag="tmp", name="rope_rot")
                nc.tensor.matmul(rot[:], perm_t[:], raw[:], start=True,
                                 stop=True)
                t1 = stg.tile([128, 512], F32, tag="stg", name="rope_t1")
                nc.vector.tensor_mul(t1[:], raw[:].bitcast(F32),
                                     cos_t[:, 512 * sc:512 * (sc + 1)])
                t2 = stg.tile([128, 512], F32, tag="stg", name="rope_t2")
                nc.vector.tensor_mul(t2[:], rot[:],
                                     sin_t[:, 512 * sc:512 * (sc + 1)])
                nc.vector.tensor_add(dest[:, 512 * sc:512 * (sc + 1)],
                                     t1[:], t2[:])

            for sc in range(SC):
                qps = [psA.tile([128, 512], F32, tag="acc", name=f"qps{h}")
                       for h in range(QH)]
                kps = psA.tile([128, 512], F32, tag="acc", name="kps")
                vps = psA.tile([128, 512], F32, tag="acc", name="vps")
                for d_i in range(DT):
                    xt = xpool.tile([128, 512], F32R, tag="x", name="xt")
                    nc.sync.dma_start(
                        xt[:], xT_d[128 * d_i:128 * (d_i + 1),
                                    512 * sc:512 * (sc + 1)])
                    wkvt = wkvp.tile([128, 2 * HD], F32R, tag="wkv",
                                     name="wkvt")
                    nc.sync.dma_start(wkvt[:],
                                      wkv_d[128 * d_i:128 * (d_i + 1), :])
                    st = d_i == 0
                    sp = d_i == DT - 1
                    for h in range(QH):
                        nc.tensor.matmul(
                            qps[h][:],
                            wq_tiles[d_i][:, HD * h:HD * (h + 1)],
                            xt[:], start=st, stop=sp)
                    nc.tensor.matmul(kps[:], wkvt[:, 0:HD], xt[:],
                                     start=st, stop=sp)
                    nc.tensor.matmul(vps[:], wkvt[:, HD:2 * HD], xt[:],
                                     start=st, stop=sp)
                for h in range(QH):
                    rope(qps[h], QT[h], sc)
                rope(kps, KT, sc)
                # V: [HD, 512] chunk -> 4 PE transposes -> natural [s, HD]
                vt_sb = stg.tile([128, 512], F32R, tag="stg", name="vt_sb")
                nc.any.tensor_copy(vt_sb[:], vps[:])
                for k4 in range(4):
                    vtp = psB.tile([128, 128], F32R, tag="tmp", name="vtp")
                    nc.tensor.transpose(vtp[:],
                                        vt_sb[:, 128 * k4:128 * (k4 + 1)],
                                        ident_t[:])
                    j = 4 * sc + k4
                    nc.any.tensor_copy(Vn[:, 128 * j:128 * (j + 1)],
                                       vtp[:].bitcast(F32))

            # ---- phase 2: attention per (q head, i-chunk) ----
            OT = [None] * QH
            for h in range(QH):
                OT[h] = headp.tile([HD, S], F32R, tag="hb", name=f"ot{h}")
                for c in range(SC):
                    qch = QT[h][:, 512 * c:512 * (c + 1)]
                    ot = psA.tile([128, 512], F32, tag="acc", name="ot_ps")
                    dacc = dap.tile([128, 512], F32R, tag="dacc",
                                    name="dacc")
                    njt = 4 * c + 4
                    for jt in range(njt):
                        stp = psA.tile([128, 512], F32, tag="acc",
                                       name="stp")
                        nc.tensor.matmul(stp[:],
                                         KT[:, 128 * jt:128 * (jt + 1)],
                                         qch, start=True, stop=True)
                        pt = ptp.tile([128, 512], F32R, tag="pt", name="pt")
                        m = jt - 4 * c
                        if m >= 0:
                            sm = stg.tile([128, 512], F32, tag="stg",
                                          name="sm")
                            nc.vector.tensor_add(sm[:], stp[:],
                                                 mask_t[m][:])
                            nc.scalar.activation(pt[:], sm[:], AF.Exp)
                        else:
                            nc.scalar.activation(pt[:], stp[:], AF.Exp)
                        nc.tensor.matmul(ot[:],
                                         Vn[:, 128 * jt:128 * (jt + 1)],
                                         pt[:], start=(jt == 0),
                                         stop=(jt == njt - 1))
                        if jt == 0:
                            nc.vector.tensor_copy(dacc[:],
                                                  pt[:].bitcast(F32))
                        else:
                            nc.vector.tensor_add(dacc[:],
                                                 dacc[:].bitcast(F32),
                                                 pt[:].bitcast(F32))
                    dsum = psB.tile([1, 512], F32, tag="tmp", name="dsum")
                    nc.tensor.matmul(dsum[:], onr_t[:], dacc[:],
                                     start=True, stop=True)
                    rc = rcp.tile([1, 512], F32R, tag="rc", name="rc")
                    with nc.allow_low_precision(reason="fp22 softmax recip"):
                        nc.vector.reciprocal(rc[:], dsum[:])
                    bc = psB.tile([128, 512], F32, tag="tmp", name="bc")
                    nc.tensor.matmul(bc[:], onb_t[:], rc[:], start=True,
                                     stop=True)
                    bcs = stg.tile([128, 512], F32, tag="stg", name="bcs")
                    nc.any.tensor_copy(bcs[:], bc[:])
                    nc.vector.tensor_mul(OT[h][:, 512 * c:512 * (c + 1)],
                                         ot[:], bcs[:])

            # ---- phase 3: output projection (row-parallel partial) ----
            for dc in range(8):
                wot = []
                for hh in range(QH):
                    w = wpool.tile([128, 512], F32R, tag="w",
                                   name=f"wo{dc}_{hh}")
                    nc.sync.dma_start(
                        w[:], wo_d[128 * hh:128 * (hh + 1),
                                   512 * dc:512 * (dc + 1)])
                    wot.append(w)
                for st_i in range(JT):
                    acc = psA.tile([128, 512], F32, tag="acc", name="oacc")
                    for hh in range(QH):
                        nc.tensor.matmul(
                            acc[:],
                            OT[hh][:, 128 * st_i:128 * (st_i + 1)],
                            wot[hh][:], start=(hh == 0), stop=(hh == QH - 1))
                    osb = stg.tile([128, 512], F32, tag="stg", name="osb")
                    nc.any.tensor_copy(osb[:], acc[:])
                    nc.sync.dma_start(
                        out_d[128 * st_i:128 * (st_i + 1),
                              512 * dc:512 * (dc + 1)], osb[:])

    nc.compile()
    return nc


def _host_tables():
    pos = np.arange(S, dtype=np.float64)
    inv_freq = ROPE_BASE ** (-np.arange(0, HD, 2, dtype=np.float64) / HD)
    ang = np.outer(pos, inv_freq)  # [S, HD/2]
    cos = np.cos(ang).T.astype(np.float32)  # [HD/2, S]
    sin = np.sin(ang).T.astype(np.float32)
    cos2 = np.repeat(cos, 2, axis=0)  # [HD, S]
    sin2 = np.repeat(sin, 2, axis=0)
    sin2[0::2, :] *= -1.0  # even rows get -sin, odd rows +sin

    perm = np.zeros((HD, HD), dtype=np.float32)
    for i in range(HD):
        perm[i ^ 1, i] = 1.0

    masks = np.zeros((4, 128, 512), dtype=np.float32)
    jr = np.arange(128)[:, None]
    ir = np.arange(512)[None, :]
    for m in range(4):
        masks[m] = np.where(jr + 128 * m <= ir, 0.0, NEG)

    return cos2, sin2, perm, masks


def kernel(x, Wq, Wk, Wv, Wo):
    global LAST_EXEC_NS
    if "nc" not in _CACHE:
        _CACHE["nc"] = _build_nc()
    nc = _CACHE["nc"]

    x = np.asarray(x, dtype=np.float32).reshape(S, D)
    Wq = np.asarray(Wq, dtype=np.float32)
    Wk = np.asarray(Wk, dtype=np.float32)
    Wv = np.asarray(Wv, dtype=np.float32)
    Wo = np.asarray(Wo, dtype=np.float32)

    xT = np.ascontiguousarray(x.T)  # [D, S]
    cos2, sin2, perm, masks = _host_tables()
    scale = np.float32(1.0 / np.sqrt(HD))
    ident = np.eye(HD, dtype=np.float32)
    ones_red = np.ones((128, 1), dtype=np.float32)
    ones_bc = np.ones((1, 128), dtype=np.float32)

    in_maps = []
    for c in range(NCORES):
        qs = slice(QH * HD * c, QH * HD * (c + 1))
        ks = slice(HD * c, HD * (c + 1))
        in_maps.append({
            "xT": xT,
            "wq": np.ascontiguousarray(Wq[:, qs]) * scale,
            "wkv": np.ascontiguousarray(
                np.concatenate([Wk[:, ks], Wv[:, ks]], axis=1)),
            "wo": np.ascontiguousarray(Wo[qs, :]),
            "cos2": cos2,
            "sin2": sin2,
            "perm": perm,
            "masks": masks,
            "ones_red": ones_red,
            "ones_bc": ones_bc,
            "ident": ident,
        })

    res = run_bass_kernel_spmd(nc, in_maps, list(range(NCORES)),
                               trace=TRACE)
    LAST_EXEC_NS = res.exec_time_ns

    out = res.results[0]["out"].astype(np.float32)
    for c in range(1, NCORES):
        out = out + res.results[c]["out"]
    return out.reshape(1, S, D)


# revision 37
# speedup vs baseline: 1.2072x; 1.2072x over previous
"""Tensor-parallel causal GQA self-attention (B=1, S=2048, D=4096, 32 q heads /
8 kv heads, HD=128, interleaved RoPE) on 8 trn2 NeuronCores.

Sharding: core c owns kv head c and q heads 4c..4c+3 (column-parallel
Wq/Wk/Wv, row-parallel Wo).  Each core computes a full [S, D] partial of the
output projection; the host sums the 8 partials (the "all-reduce").

On-device layout strategy (all matmuls fp32r = fp22 multiply at full PE rate):
  QT/KT  [HD, S]   "transposed" per-head layouts straight out of the
                   projection matmuls (lhsT = W d-tile, rhs = xT d-tile)
  RoPE   applied in [HD, S] layout: rope(q) = q*C2 + perm(q)*S2, where
                   perm is a 128x128 pair-swap matmul and C2/S2 host tables
  scores ST[j, i] = KT_tile.T @ QT_chunk  (single K=128 pass, N=512)
  P      exp(ST) via ScalarE straight from PSUM; causal masking is a
         multiplicative 0/1 mask on DVE afterwards (exp(s+m) = exp(s)*bm)
  AV     OT[h, i] += V_tile.T(natural) @ PT_tile  (fp32r, N=512)
  denom  per-j-tile ones-row matmul accumulated in a [1,512] PSUM bank,
         reciprocal on DVE, broadcast back to 128 partitions by a K=1 matmul
  o_proj out[s, dout] += OTn_tile.T @ Wo_tile  (fp32r, N=512)

DMA-issue cost (~1.26us per dma_start regardless of size) dominates naive
streaming, so transfers are batched via host-side interleaved layouts:
  xw   [4, 16, 128, 1536]  x-chunk + wkv for TWO d-tiles per DMA
  wq2  [16, 128, 1024]     two wq d-tiles per DMA (ACT hwdge ring)
  wo4  [8, 128, 2048]      all four head-blocks of a Wo dout-chunk, 2 DMAs
  out  [8, 128, 8192]      staged [128,1024] stores, host re-assembles
"""

import sys

if "/opt/trn_rl_repo" not in sys.path:
    sys.path.insert(0, "/opt/trn_rl_repo")

import numpy as np

import concourse.bass as bass
import concourse.tile as tile
from concourse import bacc, mybir
from concourse.bass_utils import run_bass_kernel_spmd

S, D, NH, NKV, HD = 2048, 4096, 32, 8, 128
NCORES = 8
QH = NH // NCORES  # 4 q heads per core
ROPE_BASE = 500000.0
NEG = -1e30

F32 = mybir.dt.float32
F32R = mybir.dt.float32r
AF = mybir.ActivationFunctionType

SC = S // 512   # 4 s-chunks of 512
DP = D // 256   # 16 d-tile pairs
JT = S // 128   # 16 j-tiles of 128

_CACHE = {}

# set by test harness to collect an exec-time profile
TRACE = False
LAST_EXEC_NS = None


def _build_nc():
    nc = bacc.Bacc("TRN2", target_bir_lowering=False, debug=False,
                   num_devices=NCORES)

    xw_d = nc.declare_dram_parameter("xw", [SC, DP, 128, 1536], F32R,
                                     isOutput=False)
    wq_d = nc.declare_dram_parameter("wq2", [DP, 128, 1024], F32R,
                                     isOutput=False)
    wo_d = nc.declare_dram_parameter("wo4", [8, 2, 128, 1024], F32R,
                                     isOutput=False)
    cos_d = nc.declare_dram_parameter("cos2", [SC, HD, 512], F32, isOutput=False)
    sin_d = nc.declare_dram_parameter("sin2", [SC, HD, 512], F32, isOutput=False)
    perm_d = nc.declare_dram_parameter("perm", [HD, HD], F32R, isOutput=False)
    masks_d = nc.declare_dram_parameter("masks", [4, 128, 512], F32,
                                        isOutput=False)
    onr_d = nc.declare_dram_parameter("ones_red", [128, 1], F32R,
                                      isOutput=False)
    onb_d = nc.declare_dram_parameter("ones_bc", [1, 128], F32R,
                                      isOutput=False)
    ident_d = nc.declare_dram_parameter("ident", [HD, HD], F32R,
                                        isOutput=False)
    out_d = nc.declare_dram_parameter("out", [8, 8, 128, 1024], F32,
                                      isOutput=True)

    with tile.TileContext(nc) as tc:
        from contextlib import ExitStack
        ctx = ExitStack()
        with ctx:
            wpool = ctx.enter_context(tc.tile_pool(name="wpool", bufs=16))
            xpool = ctx.enter_context(tc.tile_pool(name="xpool", bufs=3))
            qtp = ctx.enter_context(tc.tile_pool(name="qtp", bufs=9))
            otnp = ctx.enter_context(tc.tile_pool(name="otnp", bufs=16))
            tabp = ctx.enter_context(tc.tile_pool(name="tabp", bufs=4))
            ktp = ctx.enter_context(tc.tile_pool(name="ktp", bufs=4))
            vnp = ctx.enter_context(tc.tile_pool(name="vnp", bufs=4))
            stg = ctx.enter_context(tc.tile_pool(name="stg", bufs=4))
            rawp = ctx.enter_context(tc.tile_pool(name="rawp", bufs=6))
            ptp = ctx.enter_context(tc.tile_pool(name="ptp", bufs=4))
            mkp = ctx.enter_context(tc.tile_pool(name="mkp", bufs=4))
            cst = ctx.enter_context(tc.tile_pool(name="cst", bufs=1))
            rcp = ctx.enter_context(tc.tile_pool(name="rcp", bufs=1))
            ostp = ctx.enter_context(tc.tile_pool(name="ostp", bufs=3))
            psA = ctx.enter_context(
                tc.tile_pool(name="psA", bufs=6, space=bass.MemorySpace.PSUM))
            psB = ctx.enter_context(
                tc.tile_pool(name="psB", bufs=2, space=bass.MemorySpace.PSUM))

            # small tables, spread one DMA per d-pair across both hwdge rings
            perm_t = cst.tile([HD, HD], F32R, name="perm_t")
            ident_t = cst.tile([HD, HD], F32R, name="ident_t")
            onr_t = cst.tile([128, 1], F32R, name="onr_t")
            onb_t = cst.tile([1, 128], F32R, name="onb_t")
            mask_t = [mkp.tile([128, 512], F32, tag="mask", name=f"mask{m}")
                      for m in range(4)]

            def table_loads():
                yield lambda: nc.sync.dma_start(perm_t[:], perm_d[:])
                yield lambda: nc.scalar.dma_start(ident_t[:], ident_d[:])
                yield lambda: nc.sync.dma_start(onr_t[:], onr_d[:])
                yield lambda: nc.scalar.dma_start(onb_t[:], onb_d[:])
                for m in range(4):
                    eng = nc.sync if m % 2 == 0 else nc.scalar
                    yield lambda m=m, eng=eng: eng.dma_start(
                        mask_t[m][:], masks_d[m])

            wq_tiles = [None] * DP

            # persistent activations, one tile per (tensor, s-chunk) so a
            # reader depends only on its own chunk's producer
            QTc = [[qtp.tile([HD, 512], F32R, tag="qtc", name=f"qt{h}_{c}")
                    for c in range(SC)] for h in range(QH)]
            KTc = [ktp.tile([HD, 512], F32R, tag="ktc", name=f"kt{c}")
                   for c in range(SC)]
            Vnc = [vnp.tile([128, 512], F32R, tag="vnc", name=f"vn{c}")
                   for c in range(SC)]

            # ---- phase 1: QKV projections + RoPE + V transpose ----
            # At each s-chunk boundary, drain all six PSUM accumulators with
            # copies first (split across ACT/DVE) so the next chunk's
            # accumulation can begin; the PE work (perm matmuls, transposes)
            # and DVE combines are deferred into the next chunk's d-loop (or
            # the first attention chunk after sc=3).
            def rope_copy(acc_ps, eng):
                raw = rawp.tile([128, 512], F32R, tag="raw", name="rope_raw")
                if eng == "act":
                    nc.scalar.activation(raw[:], acc_ps[:], AF.Copy)
                else:
                    nc.vector.tensor_copy(raw[:], acc_ps[:])
                return raw

            def rope_combine(raw, dest, sc, cc, sn):
                rot = psB.tile([128, 512], F32, tag="tmp", name="rope_rot")
                nc.tensor.matmul(rot[:], perm_t[:], raw[:], start=True,
                                 stop=True)
                t1 = stg.tile([128, 512], F32, tag="stg", name="rope_t1")
                nc.vector.tensor_mul(t1[:], raw[:].bitcast(F32), cc[:])
                t2 = stg.tile([128, 512], F32, tag="stg", name="rope_t2")
                nc.vector.tensor_mul(t2[:], rot[:], sn[:])
                nc.vector.tensor_add(dest[:], t1[:], t2[:])

            boundary_pe = [None]

            def emit_boundary_pe():
                if boundary_pe[0] is not None:
                    boundary_pe[0]()
                    boundary_pe[0] = None

            cs_tiles = [None] * SC  # (cos, sin) chunk tiles, single-use

            for sc in range(SC):
                qps = [psA.tile([128, 512], F32, tag="acc", name=f"qps{h}")
                       for h in range(QH)]
                kps = psA.tile([128, 512], F32, tag="acc", name="kps")
                vps = psA.tile([128, 512], F32, tag="acc", name="vps")
                for dp in range(DP):
                    if sc == 0:
                        wt = wpool.tile([128, 1024], F32R, tag="w",
                                        name=f"wq{dp}")
                        nc.scalar.dma_start(wt[:], wq_d[dp])
                        wq_tiles[dp] = wt
                    xt = xpool.tile([128, 1536], F32R, tag="x", name="xt")
                    nc.sync.dma_start(xt[:], xw_d[sc, dp])
                    if sc == 0:
                        if dp == 0:
                            _tl = table_loads()
                        next(_tl, lambda: None)()
                    if dp == 1:
                        emit_boundary_pe()
                    if dp == 8:
                        # prefetch this boundary's cos/sin chunk
                        cc = tabp.tile([128, 512], F32, tag="tab", name="cc")
                        nc.sync.dma_start(cc[:], cos_d[sc])
                        sn = tabp.tile([128, 512], F32, tag="tab", name="sn")
                        nc.sync.dma_start(sn[:], sin_d[sc])
                        cs_tiles[sc] = (cc, sn)
                    for half in range(2):
                        xs = xt[:, 768 * half:768 * half + 512]
                        ks = xt[:, 768 * half + 512:768 * half + 640]
                        vs = xt[:, 768 * half + 640:768 * half + 768]
                        st = dp == 0 and half == 0
                        sp = dp == DP - 1 and half == 1
                        wqh = wq_tiles[dp][:, 512 * half:512 * (half + 1)]
                        for h in range(QH):
                            nc.tensor.matmul(
                                qps[h][:], wqh[:, HD * h:HD * (h + 1)],
                                xs, start=st, stop=sp)
                        # the wkv columns ride along in the same xw tile and
                        # serve as the stationary operand for k/v
                        nc.tensor.matmul(kps[:], ks, xs, start=st, stop=sp)
                        nc.tensor.matmul(vps[:], vs, xs, start=st, stop=sp)

                raw_k = rope_copy(kps, "act")
                vt_sb = rope_copy(vps, "dve")
                raw_q = [None] * QH
                raw_q[0] = rope_copy(qps[0], "dve")

                def boundary(sc=sc, raw_k=raw_k, vt_sb=vt_sb, raw_q=raw_q):
                    cc, sn = cs_tiles[sc]
                    rope_combine(raw_k, KTc[sc], sc, cc, sn)
                    rope_combine(raw_q[0], QTc[0][sc], sc, cc, sn)
                    for k4 in range(4):
                        vtp = psB.tile([128, 128], F32R, tag="tmp",
                                       name="vtp")
                        nc.tensor.transpose(
                            vtp[:], vt_sb[:, 128 * k4:128 * (k4 + 1)],
                            ident_t[:])
                        nc.scalar.activation(
                            Vnc[sc][:, 128 * k4:128 * (k4 + 1)],
                            vtp[:].bitcast(F32), AF.Copy)
                    for h in range(1, QH):
                        rope_combine(raw_q[h], QTc[h][sc], sc, cc, sn)

                boundary_pe[0] = boundary
                if sc >= 1:
                    emit_attn[0](sc - 1)
                for h in range(1, QH):
                    raw_q[h] = rope_copy(qps[h],
                                         "act" if h % 2 == 0 else "dve")
                if sc == SC - 1:
                    # prefetch all Wo tiles now: the 16 wq slots free progressively
                    # through the last QKV chunk and the rings are otherwise idle
                    wo_tiles = []
                    for dc in range(8):
                        woa = wpool.tile([128, 1024], F32R, tag="w", name=f"woa{dc}")
                        nc.sync.dma_start(woa[:], wo_d[dc, 0])
                        wob = wpool.tile([128, 1024], F32R, tag="w", name=f"wob{dc}")
                        nc.sync.dma_start(wob[:], wo_d[dc, 1])
                        wo_tiles.append((woa, wob))

                    emit_boundary_pe()
                    emit_attn[0](SC - 1)

            # ---- phase 2: attention, interleaved with QKV by chunk ----
            # chunk c's inputs (KTc/Vnc/QTc[*][c]) are complete right after
            # s-chunk boundary c, so attention for chunk c-1 is emitted after
            # the copies of boundary c — it fills the boundary drain and its
            # DMA-free PE stream lets the next d-loop's xw prefetch run ahead.
            # The normalization tail is emitted one head late so its PE
            # instructions never stall the in-order PE queue on the DVE chain.
            otn_all = [[None] * QH for _ in range(SC)]
            tails = []

            def make_tail(otn, ot, dsum):
                def tail():
                    rc = rcp.tile([1, 512], F32R, tag="rc", name="rc")
                    with nc.allow_low_precision(reason="fp22 softmax recip"):
                        nc.vector.reciprocal(rc[:], dsum[:])
                    bc = psA.tile([128, 512], F32, tag="acc", name="bc")
                    nc.tensor.matmul(bc[:], onb_t[:], rc[:], start=True,
                                     stop=True)
                    bcs = stg.tile([128, 512], F32, tag="stg", name="bcs")
                    nc.scalar.activation(bcs[:], bc[:], AF.Copy)
                    nc.vector.tensor_mul(otn[:], ot[:], bcs[:])
                return tail

            def attn_chunk(c):
                for h in range(QH):
                    qch = QTc[h][c][:]
                    ot = psA.tile([128, 512], F32, tag="acc", name="ot_ps")
                    dsum = psB.tile([1, 512], F32, tag="tmp", name="dsum")
                    njt = 4 * c + 4
                    pts = [None] * njt

                    def score(jt, c=c, qch=qch):
                        stp = psA.tile([128, 512], F32, tag="acc",
                                       name="stp")
                        nc.tensor.matmul(
                            stp[:],
                            KTc[jt // 4][:, 128 * (jt % 4):128 * (jt % 4 + 1)],
                            qch, start=True, stop=True)
                        pt = ptp.tile([128, 512], F32R, tag="pt", name="pt")
                        m = jt - 4 * c
                        if m >= 0:
                            # exp(s+m) == exp(s)*binmask keeps ACT (the
                            # attention bottleneck) reading PSUM directly
                            pe = stg.tile([128, 512], F32, tag="stg",
                                          name="pe_t")
                            nc.scalar.activation(pe[:], stp[:], AF.Exp)
                            nc.vector.tensor_mul(pt[:], pe[:],
                                                 mask_t[m][:])
                        else:
                            nc.scalar.activation(pt[:], stp[:], AF.Exp)
                        return pt

                    def accum(jt, pt, ot=ot, dsum=dsum, njt=njt):
                        nc.tensor.matmul(
                            ot[:],
                            Vnc[jt // 4][:, 128 * (jt % 4):128 * (jt % 4 + 1)],
                            pt[:], start=(jt == 0),
                            stop=(jt == njt - 1))
                        nc.tensor.matmul(dsum[:], onr_t[:], pt[:],
                                         start=(jt == 0),
                                         stop=(jt == njt - 1))

                    for jt in range(njt):
                        pts[jt] = score(jt)
                        if jt >= 3:
                            accum(jt - 3, pts[jt - 3])
                    for k in (3, 2, 1):
                        accum(njt - k, pts[njt - k])
                    otn = otnp.tile([128, 512], F32R, tag="otn", name="otn")
                    otn_all[c][h] = otn
                    tails.append(make_tail(otn, ot, dsum))
                    if len(tails) > 1:
                        tails.pop(0)()
                # the last head's tail must not leak into the next d-loop:
                # its psum accumulators would starve the six projection
                # accumulators (only 8 banks exist)
                while tails:
                    tails.pop(0)()

            emit_attn = [attn_chunk]  # referenced from the QKV loop below

            # run the interleaved QKV + attention schedule
            for c in range(SC - 1):
                pass  # (structure only; emission happens in the QKV loop)

            # ---- phase 3: output projection (row-parallel partial) ----
            def o_proj_all():
                for dc in range(8):
                    woa, wob = wo_tiles[dc]
                    wsl = [woa[:, 0:512], woa[:, 512:1024],
                           wob[:, 0:512], wob[:, 512:1024]]
                    for c in range(SC):
                        for lp in range(2):
                            ost = ostp.tile([128, 1024], F32, tag="ost",
                                            name="ost")
                            for k2 in range(2):
                                kk = 2 * lp + k2
                                acc = psA.tile([128, 512], F32, tag="acc",
                                               name="oacc")
                                for hh in range(QH):
                                    nc.tensor.matmul(
                                        acc[:],
                                        otn_all[c][hh][:, 128 * kk:
                                                       128 * (kk + 1)],
                                        wsl[hh], start=(hh == 0),
                                        stop=(hh == QH - 1))
                                dstc = ost[:, 512 * k2:512 * (k2 + 1)]
                                if k2 == 0:
                                    nc.vector.tensor_copy(dstc, acc[:])
                                else:
                                    nc.scalar.activation(dstc, acc[:],
                                                         AF.Copy)
                            eng = nc.sync if lp == 0 else nc.scalar
                            eng.dma_start(out_d[dc, 2 * c + lp], ost[:])

            o_proj_all()

    nc.compile()
    return nc


def _host_tables():
    pos = np.arange(S, dtype=np.float64)
    inv_freq = ROPE_BASE ** (-np.arange(0, HD, 2, dtype=np.float64) / HD)
    ang = np.outer(pos, inv_freq)  # [S, HD/2]
    cos = np.cos(ang).T.astype(np.float32)  # [HD/2, S]
    sin = np.sin(ang).T.astype(np.float32)
    cos2 = np.repeat(cos, 2, axis=0)  # [HD, S]
    sin2 = np.repeat(sin, 2, axis=0)
    sin2[0::2, :] *= -1.0  # even rows get -sin, odd rows +sin

    perm = np.zeros((HD, HD), dtype=np.float32)
    for i in range(HD):
        perm[i ^ 1, i] = 1.0

    masks = np.zeros((4, 128, 512), dtype=np.float32)
    jr = np.arange(128)[:, None]
    ir = np.arange(512)[None, :]
    for m in range(4):
        masks[m] = np.where(jr + 128 * m <= ir, 1.0, 0.0)

    return cos2, sin2, perm, masks


def kernel(x, Wq, Wk, Wv, Wo):
    global LAST_EXEC_NS
    if "nc" not in _CACHE:
        _CACHE["nc"] = _build_nc()
    nc = _CACHE["nc"]

    x = np.asarray(x, dtype=np.float32).reshape(S, D)
    Wq = np.asarray(Wq, dtype=np.float32)
    Wk = np.asarray(Wk, dtype=np.float32)
    Wv = np.asarray(Wv, dtype=np.float32)
    Wo = np.asarray(Wo, dtype=np.float32)

    xT = np.ascontiguousarray(x.T)  # [D, S]
    xTr = xT.reshape(DP * 2, 128, S)
    cos2, sin2, perm, masks = _host_tables()
    cos2c = np.ascontiguousarray(cos2.reshape(HD, SC, 512).transpose(1, 0, 2))
    sin2c = np.ascontiguousarray(sin2.reshape(HD, SC, 512).transpose(1, 0, 2))
    scale = np.float32(1.0 / np.sqrt(HD))
    ident = np.eye(HD, dtype=np.float32)
    ones_red = np.ones((128, 1), dtype=np.float32)
    ones_bc = np.ones((1, 128), dtype=np.float32)

    in_maps = []
    for c in range(NCORES):
        qs = slice(QH * HD * c, QH * HD * (c + 1))
        ks = slice(HD * c, HD * (c + 1))
        wkv = np.concatenate([Wk[:, ks], Wv[:, ks]], axis=1)  # [D, 256]
        wkvr = wkv.reshape(DP * 2, 128, 256)
        # xw[sc, dp, p, :] = [x_A | wkv_A | x_B | wkv_B] for d-tile pair
        xw = np.empty((SC, DP, 128, 1536), dtype=np.float32)
        for sc in range(SC):
            cs = slice(512 * sc, 512 * (sc + 1))
            xw[sc, :, :, 0:512] = xTr[0::2, :, cs]
            xw[sc, :, :, 512:768] = wkvr[0::2]
            xw[sc, :, :, 768:1280] = xTr[1::2, :, cs]
            xw[sc, :, :, 1280:1536] = wkvr[1::2]

        wqc = (Wq[:, qs] * scale).reshape(DP * 2, 128, 512)
        wq2 = np.concatenate([wqc[0::2], wqc[1::2]], axis=2)  # [DP,128,1024]

        # wo4[dc, half, p, 512*(hh%2)+col] = Wo[128*(2*half+hh%2)+p,
        #                                        512*dc+col]
        wo4 = np.ascontiguousarray(
            Wo[qs, :].reshape(2, 2, 128, 8, 512).transpose(3, 0, 2, 1, 4)
            .reshape(8, 2, 128, 1024))

        in_maps.append({
            "xw": xw,
            "wq2": np.ascontiguousarray(wq2),
            "wo4": wo4,
            "cos2": cos2c,
            "sin2": sin2c,
            "perm": perm,
            "masks": masks,
            "ones_red": ones_red,
            "ones_bc": ones_bc,
            "ident": ident,
        })

    res = run_bass_kernel_spmd(nc, in_maps, list(range(NCORES)),
                               trace=TRACE)
    LAST_EXEC_NS = res.exec_time_ns

    acc = res.results[0]["out"].astype(np.float32)
    for c in range(1, NCORES):
        acc = acc + res.results[c]["out"]
    # out[dc, sp2, p, k2*512 + col] -> out[(2*sp2+k2)*128 + p, dc*512 + col]
    out = (acc.reshape(8, 8, 128, 2, 512).transpose(1, 3, 2, 0, 4)
           .reshape(S, D))
    return np.ascontiguousarray(out).reshape(1, S, D)
